# revision 1
# baseline (speedup 1.0000x reference)
"""CrossLinear attention kernel for Trainium2 (8 NeuronCores, data-parallel over batch).

Computes, per batch element b:
    scores = x_b @ x_b^T            [T, T]
    scores[mask] = -inf
    attn = softmax(scores, axis=-1)
    xx = x_b @ W                    [T, C]
    out_b = attn @ xx               [T, C]

with B=8, T=2048, C=1024 (fp32).  One batch element per NeuronCore.

Design notes:
  - All big matmuls use float32r operands -> 1 cycle/row on the PE (plain
    fp32 is 4 cycles/row).  f32r is a rounded fp32 format (TF32-like): the
    BIR verifier requires every producer feeding an f32r matmul to emit
    f32r, so all matmul-input tiles are allocated f32r and filled by
    on-chip compute (DVE/ACT copies round).  End-to-end rel err ~6e-4.
  - x is transposed once on the PE into xT [C, T] layout; both score-matmul
    operands and the x@W lhsT come from xT.
  - Masking is applied in PSUM via copy_predicated (mask u8, -1e9 fill),
    per 512-column chunk, interleaved with the score matmuls of the
    following chunks (no serial DVE tail).
  - Softmax: per-chunk DVE row max -> combine (negated) -> ACT exp with
    per-partition bias and accumulated row sum -> 1/sum folded into the
    final output scale (out = (attn_unnorm @ xx) * 1/sum).
  - attn is transposed 128x128 at a time on the PE so it can serve as lhsT
    of the attn @ xx matmul; psum->sbuf casts go to the scalar engine to
    keep the DVE queue shallow.
  - Software pipeline: emit scores(i+1); transposes(i); softmax(i+1);
    out(i) so each engine queue stays covered:
      PE : scores(i+1), T(i), out(i), scores(i+2), ...
      DVE: pred/max chunks interleaved with the score matmuls
      ACT: pT casts(i), exp(i+1), out-scale(i)
  - This container's walrus accepts at most ONE sync-wait per instruction;
    _split_sync_waits hoists extras onto single-wait NoOps.
"""

import sys

if "/opt/trn_rl_repo" not in sys.path:
    sys.path.insert(0, "/opt/trn_rl_repo")

from contextlib import ExitStack

import numpy as np

import concourse.bass as bass
import concourse.mybir as mybir
import concourse.tile as tile
from concourse import bass_utils
from concourse.bass import ds, ts
from concourse.masks import make_identity

B, T, C = 8, 2048, 1024
P = 128                 # partition block
NT = T // P             # 16 row blocks
NKC = C // P            # 8 contraction chunks over C
NKS = T // P            # 16 contraction chunks over T (for attn @ xx)
NEG_BIG = -1.0e9

F32 = mybir.dt.float32
F32R = mybir.dt.float32r
U8 = mybir.dt.uint8


def build_bass():
    nc = bass.Bass(
        trn_type="TRN2",
        target_bir_lowering=False,
        debug=False,
        enable_asserts=False,
        num_devices=8,
    )
    x_d = nc.dram_tensor("x", [T, C], F32, kind="ExternalInput").ap()
    m_d = nc.dram_tensor("mask", [T, T], U8, kind="ExternalInput").ap()
    w_d = nc.dram_tensor("W", [C, C], F32, kind="ExternalInput").ap()
    o_d = nc.dram_tensor("out", [T, C], F32, kind="ExternalOutput").ap()

    with tile.TileContext(nc) as tc:
        _kernel_body(nc, tc, x_d, m_d, w_d, o_d)
    return nc


def _kernel_body(nc, tc, x_d, m_d, w_d, o_d):
    with ExitStack() as big:
        const = big.enter_context(tc.tile_pool(name="const", bufs=1))
        ident = const.tile([P, P], F32)
        make_identity(nc, ident[:])
        ident_r = const.tile([P, P], F32R)
        nc.vector.tensor_copy(ident_r[:], ident[:])
        negbig = const.tile([P, T], F32)
        nc.gpsimd.memset(negbig[:], NEG_BIG)

        persist = big.enter_context(tc.tile_pool(name="persist", bufs=1))
        xT = persist.tile([P, NKC, T], F32R)   # xT[p, k, t] = x[t, k*128+p]
        xx = persist.tile([P, NT, C], F32R)    # xx[p, i, d] = (x@W)[i*128+p, d]

        # ---- Phase 1: transpose x into xT; compute xx = x @ W ----
        with tc.tile_pool(name="wpool", bufs=1) as wpool, \
             tc.tile_pool(name="wstage", bufs=2) as wstage, \
             tc.tile_pool(name="xload", bufs=3) as xload, \
             tc.tile_pool(name="psA", bufs=2, space="PSUM") as psA, \
             tc.tile_pool(name="psXX", bufs=2, space="PSUM") as psXX:

            w_sb = wpool.tile([P, NKC, C], F32R)
            for k in range(NKC):
                w_st = wstage.tile([P, C], F32, tag="wst")
                # W[k*128+p, d] -> w_st[p, d]
                nc.sync.dma_start(w_st[:], w_d[ts(k, P), :])
                nc.scalar.copy(w_sb[:, k, :], w_st[:])

            def xpose(i):
                xt_i = xload.tile([P, C], F32, tag="xt")
                nc.sync.dma_start(xt_i[:], x_d[ts(i, P), :])
                for g in range(2):
                    pt = psA.tile([P, 4, P], F32, tag="pt")
                    for j in range(4):
                        k = g * 4 + j
                        nc.tensor.transpose(
                            pt[:, j, :], xt_i[:, ds(k * P, P)], ident[:]
                        )
                    nc.vector.tensor_copy(
                        xT[:, ds(g * 4, 4), ds(i * P, P)], pt[:]
                    )

            def xxmm(i):
                po = psXX.tile([P, C], F32, tag="po1")
                for k in range(NKC):
                    for h in range(2):
                        nc.tensor.matmul(
                            po[:, ds(h * 512, 512)],
                            lhsT=xT[:, k, ds(i * P, P)],
                            rhs=w_sb[:, k, ds(h * 512, 512)],
                            start=(k == 0),
                            stop=(k == NKC - 1),
                        )
                nc.vector.tensor_copy(xx[:, i, :], po[:])

            # stagger: transpose tile i+1 while xx(i) runs, so the DVE
            # casts of tile i are done by the time xx(i) needs them
            xpose(0)
            for i in range(NT):
                if i + 1 < NT:
                    xpose(i + 1)
                xxmm(i)

        # ---- Phase 2: attention main loop over row blocks ----
        with tc.tile_pool(name="maskp", bufs=2) as maskp, \
             tc.tile_pool(name="ppool", bufs=2) as ppool, \
             tc.tile_pool(name="ptpool", bufs=2) as ptpool, \
             tc.tile_pool(name="opool", bufs=2) as opool, \
             tc.tile_pool(name="stats", bufs=4) as stats, \
             tc.tile_pool(name="psS", bufs=1, space="PSUM") as psS, \
             tc.tile_pool(name="psT", bufs=2, space="PSUM") as psT, \
             tc.tile_pool(name="psO", bufs=1, space="PSUM") as psO:

            def scores(i):
                """Score matmuls for row block i.  Mask-apply (copy_predicated)
                and per-512-chunk row max run on the DVE interleaved with the
                matmuls of the following chunks, so no serial DVE tail."""
                msk = maskp.tile([P, T], U8, tag="mask")
                nc.sync.dma_start(msk[:], m_d[ts(i, P), :])
                ps = psS.tile([P, T], F32, tag="scores")
                maxpart = stats.tile([P, 4], F32, tag="maxpart")
                for n in range(4):
                    sl = ds(n * 512, 512)
                    for k in range(NKC):
                        nc.tensor.matmul(
                            ps[:, sl],
                            lhsT=xT[:, k, ds(i * P, P)],
                            rhs=xT[:, k, sl],
                            start=(k == 0),
                            stop=(k == NKC - 1),
                        )
                    nc.vector.copy_predicated(ps[:, sl], msk[:, sl], negbig[:, sl])
                    nc.vector.reduce_max(
                        maxpart[:, ds(n, 1)], ps[:, sl], axis=mybir.AxisListType.X
                    )
                return ps, maxpart

            def softmax(i, ps, maxpart):
                negmax = stats.tile([P, 1], F32, tag="negmax")
                nc.vector.reduce_max(
                    negmax[:], maxpart[:], axis=mybir.AxisListType.X, negate=True
                )
                rowsum = stats.tile([P, 1], F32, tag="rowsum")
                p_i = ppool.tile([P, T], F32R, tag="p")
                nc.scalar.activation(
                    p_i[:],
                    ps[:],
                    mybir.ActivationFunctionType.Exp,
                    bias=negmax[:],
                    scale=1.0,
                    accum_out=rowsum[:],
                )
                recip = stats.tile([P, 1], F32, tag="recip")
                nc.vector.reciprocal(recip[:], rowsum[:])
                return p_i, recip

            def pv_transpose(i, p_i):
                """PE-transpose attn weights; psum->sbuf casts on the (idle)
                scalar engine so the DVE never blocks the PE here."""
                pT = ptpool.tile([P, NKS, P], F32R, tag="pT")
                for g in range(4):
                    pt_ps = psT.tile([P, 4, P], F32R, tag="ptps")
                    for j in range(4):
                        s = g * 4 + j
                        nc.tensor.transpose(
                            pt_ps[:, j, :], p_i[:, ds(s * P, P)], ident_r[:]
                        )
                    nc.scalar.copy(pT[:, ds(g * 4, 4), :], pt_ps[:])
                return pT

            def pv_out(i, pT, recip):
                po = psO.tile([P, C], F32, tag="po2")
                for s in range(NKS):
                    for h in range(2):
                        nc.tensor.matmul(
                            po[:, ds(h * 512, 512)],
                            lhsT=pT[:, s, :],
                            rhs=xx[:, s, ds(h * 512, 512)],
                            start=(s == 0),
                            stop=(s == NKS - 1),
                        )
                out_t = opool.tile([P, C], F32, tag="out")
                nc.scalar.mul(out_t[:], po[:], recip[:])
                nc.sync.dma_start(o_d[ts(i, P), :], out_t[:])

            # software pipeline; engine-queue order is the point
            sc = scores(0)
            sm = softmax(0, *sc)
            for i in range(NT):
                nxt_sc = scores(i + 1) if i + 1 < NT else None
                pT = pv_transpose(i, sm[0])
                nxt_sm = softmax(i + 1, *nxt_sc) if nxt_sc else None
                pv_out(i, pT, sm[1])
                sm = nxt_sm


def _split_sync_waits(nc, limit: int = 1):
    """The walrus build in this container rejects instructions with more than
    one sync-wait command.  Hoist excess waits onto preceding single-wait
    NoOps on the same engine (waits execute in order before the original
    instruction, so semantics are preserved)."""
    n_new = 0
    for fn in nc.m.functions:
        for blk in fn.blocks:
            new_insts = []
            for inst in blk.instructions:
                si = inst.sync_info
                if si and si.on_wait and len(si.on_wait) > limit:
                    waits = list(si.on_wait)
                    extra, keep = waits[:-limit], waits[-limit:]
                    for w in extra:
                        nop = mybir.InstNoOp(
                            name=f"{inst.name}-wsplit{n_new}", ins=[], outs=[]
                        )
                        n_new += 1
                        nop.engine = inst.engine
                        nop.sync_info = mybir.SyncInfo(on_wait=[w], on_update=[])
                        new_insts.append(nop)
                    si.on_wait[:] = keep
                new_insts.append(inst)
            blk.instructions[:] = new_insts
    return n_new


_NC_CACHE = None


def _get_nc():
    global _NC_CACHE
    if _NC_CACHE is None:
        nc = build_bass()
        _split_sync_waits(nc, limit=1)
        _NC_CACHE = nc
    return _NC_CACHE


def run(inputs: dict, trace: bool = False, tmpdir: str | None = None):
    """Run on 8 NeuronCores; returns (out [B,T,C] f32, BassKernelResults)."""
    nc = _get_nc()
    x = np.ascontiguousarray(np.asarray(inputs["x"], dtype=np.float32))
    mask = np.asarray(inputs["mask"])
    if mask.dtype != np.uint8:
        mask = mask.astype(np.uint8)
    mask = np.ascontiguousarray(mask)
    w = np.ascontiguousarray(np.asarray(inputs["W"], dtype=np.float32))
    in_maps = [
        {"x": x[b], "mask": mask[b], "W": w} for b in range(B)
    ]
    res = bass_utils.run_bass_kernel_spmd(
        nc,
        in_maps,
        core_ids=list(range(B)),
        trace=trace,
        tmpdir=tmpdir,
    )
    out = np.stack([res.results[b]["out"] for b in range(B)], axis=0)
    return out, res


def kernel(**inputs) -> np.ndarray:
    out, _ = run(inputs, trace=False)
    return out

